# revision 8
# baseline (speedup 1.0000x reference)
"""Trainium2 Bass kernel for a pre-norm transformer block with dilated
windowed causal attention (B=2, L=2048, D=512, H=8, DIL=2, WIN=256,
HIDDEN=2048).

Sharding: 8 cores = batch(2) x sequence-chunk(4 x 512 tokens). Each core
receives its 512-token chunk plus a 256-token halo (keys/values only) and
computes the full block for its tokens; no collectives.

Device dataflow (per core):
  x [768,512] f32 -> LN1 (token-major stats) -> x_hat bf16 -> PE-transpose
  -> x_hat^T.  QKV (bf16 matmuls, fp32 psum): Q^T,K^T feature-major
  [hd, t]; V token-major per parity stream, ones-augmented per head.
  Attention per (head, parity stream, 128-query block): S^T[k,q] matmuls
  (dilation=2 -> two independent parity streams with a 128-wide causal
  window in stream coords), exp on ScalarE (no max subtraction: scores
  are O(1) here), band-mask multiply on GpSimd, PV with the ones row
  producing the softmax denominator, f32 reciprocal + gpsimd
  partition_broadcast for the normalization -> O^T feature-major bf16.
  out-proj -> residual (f32) -> LN2 -> FFN1+gelu -> FFN2 -> residual ->
  out [512,512] f32.

LayerNorm scale/bias are folded into the projection weights host-side;
weights are pre-transposed and cast to bf16 host-side.
"""
import os
import sys

os.environ.setdefault("MYCRO_LOCAL_CACHE", "1")
if "/opt/trn_rl_repo" not in sys.path:
    sys.path.insert(0, "/opt/trn_rl_repo")

import numpy as np

B, L, D, H, HD = 2, 2048, 512, 8, 64
HIDDEN = 4 * D
P = 128
CH = 512            # own tokens per core
HALO = 256
T = CH + HALO       # 768
NCORES = 8
EPS = 1e-5
SL = T // 2         # 384 keys per parity stream
SQ = CH // 2        # 256 queries per parity stream
SCALE = 1.0 / 8.0   # 1/sqrt(HD)

NT = T // P         # 6
NO = CH // P        # 4
ND = D // P         # 4
NHID = HIDDEN // P  # 16

_nc = None
LAST_EXEC_NS = None
LAST_RESULTS = None


def _body(ctx, tc, I, y):
    import concourse.bass as bass  # noqa: F401
    from concourse import mybir
    from concourse.masks import make_identity

    nc = tc.nc
    f32 = mybir.dt.float32
    bf16 = mybir.dt.bfloat16
    AF = mybir.ActivationFunctionType
    OP = mybir.AluOpType

    consts = ctx.enter_context(tc.tile_pool(name="consts", bufs=1))
    big = ctx.enter_context(tc.tile_pool(name="big", bufs=1))
    work = ctx.enter_context(tc.tile_pool(name="work", bufs=4))
    pmm = ctx.enter_context(tc.tile_pool(name="pmm", bufs=2, space="PSUM"))
    ptp = ctx.enter_context(tc.tile_pool(name="ptp", bufs=2, space="PSUM"))
    pa_s = ctx.enter_context(tc.tile_pool(name="pa_s", bufs=2, space="PSUM"))
    pa_o = ctx.enter_context(tc.tile_pool(name="pa_o", bufs=2, space="PSUM"))

    mm = nc.tensor.matmul

    def bcast(ap, p=P):
        return bass.AP(tensor=ap.tensor, offset=ap.offset,
                       ap=[[0, p]] + [list(d) for d in ap.ap])

    # ---------- constants ----------
    ident = consts.tile([P, P], bf16, tag="ident")
    make_identity(nc, ident)
    epst = consts.tile([P, 1], f32, tag="eps")
    nc.vector.memset(epst, EPS)

    masks_sb = consts.tile([P, 3, P], bf16, tag="masks")
    nc.sync.dma_start(out=masks_sb, in_=I["masks"].rearrange("k p q -> p k q"))
    bq_sb = consts.tile([P, 4], f32, tag="bq")
    nc.sync.dma_start(out=bq_sb, in_=I["bq"])
    bk_sb = consts.tile([P, 4], f32, tag="bk")
    nc.sync.dma_start(out=bk_sb, in_=I["bk"])
    b1_sb = consts.tile([P, NHID], f32, tag="b1")
    nc.sync.dma_start(out=b1_sb, in_=I["b1"])
    bv_sb = consts.tile([P, D], f32, tag="bv")
    nc.gpsimd.dma_start(out=bv_sb, in_=bcast(I["bv"]))
    bo_sb = consts.tile([P, D], f32, tag="bo")
    nc.gpsimd.dma_start(out=bo_sb, in_=bcast(I["bo"]))
    b2_sb = consts.tile([P, D], f32, tag="b2")
    nc.gpsimd.dma_start(out=b2_sb, in_=bcast(I["b2"]))

    # ---------- load x ----------
    x_sb = big.tile([P, NT, D], f32, tag="x")
    nc.sync.dma_start(out=x_sb, in_=I["xc"].rearrange("(j p) d -> p j d", p=P))

    # ---------- LN1 (token-major) ----------
    xhat = big.tile([P, NT, D], bf16, tag="t12a")
    for j in range(NT):
        st = work.tile([P, 6], f32, tag="bnst")
        nc.vector.bn_stats(st, x_sb[:, j, :])
        mv = work.tile([P, 2], f32, tag="bnmv")
        nc.vector.bn_aggr(mv, st)
        r = work.tile([P, 1], f32, tag="lnr")
        nc.scalar.activation(r, mv[:, 1:2], AF.Sqrt, bias=epst, scale=1.0)
        r2 = work.tile([P, 1], f32, tag="lnr2")
        nc.vector.reciprocal(r2, r)
        nc.vector.tensor_scalar(
            out=xhat[:, j, :], in0=x_sb[:, j, :],
            scalar1=mv[:, 0:1], scalar2=r2,
            op0=OP.subtract, op1=OP.mult,
        )

    # ---------- transpose x_hat -> x_hat^T [d, t] ----------
    xT = big.tile([P, ND, T], bf16, tag="t12b")
    for dt_ in range(ND):
        for j in range(NT):
            pt = ptp.tile([P, P], bf16, tag="pt")
            nc.tensor.transpose(pt, xhat[:, j, dt_ * P:(dt_ + 1) * P], ident)
            nc.scalar.copy(xT[:, dt_, j * P:(j + 1) * P], pt)

    # ---------- QKV ----------
    wqkv_sb = big.tile([P, ND, 3 * D], bf16, tag="w32a")
    nc.sync.dma_start(out=wqkv_sb, in_=I["wqkvT"].rearrange("(i p) o -> p i o", p=P))

    # Q^T [o, own t]  (own tokens only)
    qT = big.tile([P, 4, CH], bf16, tag="t8a")
    for ot in range(4):
        ps = pmm.tile([P, CH], f32, tag="ps")
        for dt_ in range(ND):
            mm(ps, wqkv_sb[:, dt_, ot * P:(ot + 1) * P], xT[:, dt_, HALO:],
               start=(dt_ == 0), stop=(dt_ == ND - 1))
        nc.scalar.activation(qT[:, ot, :], ps, AF.Identity,
                             bias=bq_sb[:, ot:ot + 1], scale=1.0)

    # K^T [o, all t] in chunks of 512+256 (one PSUM bank each)
    kT = big.tile([P, 4, T], bf16, tag="t12a2")
    for ot in range(4):
        for c0, cn in ((0, 512), (512, 256)):
            ps = pmm.tile([P, CH], f32, tag="ps")
            for dt_ in range(ND):
                mm(ps[:, :cn], wqkv_sb[:, dt_, (4 + ot) * P:(5 + ot) * P],
                   xT[:, dt_, c0:c0 + cn],
                   start=(dt_ == 0), stop=(dt_ == ND - 1))
            nc.scalar.activation(kT[:, ot, c0:c0 + cn], ps[:, :cn], AF.Identity,
                                 bias=bk_sb[:, ot:ot + 1], scale=1.0)

    # V token-major per parity stream, ones-augmented per head:
    # v_sb[:, st*3+i, h, 0:64] = V tokens, [..., 64] = 1.0
    v_sb = big.tile([P, 6, H, 65], bf16, tag="t12c")
    for i in range(6):
        nc.vector.memset(v_sb[:, i, :, 64:65], 1.0)
    for stp in range(2):
        for i in range(3):
            ps = pmm.tile([P, D], f32, tag="ps")
            t0 = 2 * (i * P) + stp
            for dt_ in range(ND):
                mm(ps, xT[:, dt_, t0:t0 + 255:2], wqkv_sb[:, dt_, 2 * D:3 * D],
                   start=(dt_ == 0), stop=(dt_ == ND - 1))
            nc.vector.tensor_add(
                v_sb[:, stp * 3 + i, :, 0:64],
                ps.rearrange("p (h c) -> p h c", h=H),
                bv_sb.rearrange("p (h c) -> p h c", h=H),
            )

    # ---------- attention ----------
    # masks_sb[:, 0] = lower-tri (c<=r) with halo validity, qb=0 low tile
    # masks_sb[:, 1] = lower-tri (c<=r), qb=1 low tile
    # masks_sb[:, 2] = upper-tri (c>=r), high tiles
    oT = big.tile([P, 4, CH], bf16, tag="t12b2")
    for hp in range(4):           # head pair = partition tile of qT/kT
        for stp in range(2):      # parity stream
            for hh in range(2):   # head within pair (partitions hh*64..)
                h = 2 * hp + hh
                lo, hi = hh * 64, (hh + 1) * 64
                po = pa_o.tile([P, SQ], f32, tag="po")
                for qb in range(2):
                    for t_i, kt in enumerate((qb, qb + 1)):
                        ps_s = pa_s.tile([P, P], f32, tag="ps_s")
                        k0 = 2 * (kt * P) + stp
                        q0 = 2 * (qb * P) + stp
                        mm(ps_s,
                           kT[lo:hi, hp, k0:k0 + 255:2],
                           qT[lo:hi, hp, q0:q0 + 255:2],
                           start=True, stop=True)
                        p_sb = work.tile([P, P], bf16, tag="p_sb")
                        nc.scalar.activation(p_sb, ps_s, AF.Exp, scale=SCALE)
                        midx = 2 if t_i == 1 else (0 if qb == 0 else 1)
                        nc.gpsimd.tensor_mul(p_sb, p_sb, masks_sb[:, midx, :])
                        mm(po[:65, qb * P:(qb + 1) * P],
                           v_sb[:, stp * 3 + kt, h, :], p_sb,
                           start=(t_i == 0), stop=(t_i == 1))
                rec = work.tile([1, SQ], f32, tag="rec")
                nc.vector.reciprocal(rec, po[64:65, :])
                rb = work.tile([64, SQ], f32, tag="rb")
                nc.gpsimd.partition_broadcast(rb, rec)
                nc.vector.tensor_mul(oT[lo:hi, hp, stp::2], po[:64, :], rb)

    # ---------- out projection + residual ----------
    wo_sb = big.tile([P, ND, D], bf16, tag="t8b")
    nc.sync.dma_start(out=wo_sb, in_=I["woT"].rearrange("(i p) o -> p i o", p=P))
    res1 = big.tile([P, NO, D], f32, tag="t12c2")
    for tt in range(NO):
        ps = pmm.tile([P, D], f32, tag="ps")
        for dt_ in range(ND):
            mm(ps, oT[:, dt_, tt * P:(tt + 1) * P], wo_sb[:, dt_, :],
               start=(dt_ == 0), stop=(dt_ == ND - 1))
        nc.vector.tensor_add(res1[:, tt, :], ps, x_sb[:, 2 + tt, :])
        nc.vector.tensor_add(res1[:, tt, :], res1[:, tt, :], bo_sb)

    # ---------- LN2 ----------
    xhat2 = big.tile([P, NO, D], bf16, tag="t8b2")
    for j in range(NO):
        st = work.tile([P, 6], f32, tag="bnst")
        nc.vector.bn_stats(st, res1[:, j, :])
        mv = work.tile([P, 2], f32, tag="bnmv")
        nc.vector.bn_aggr(mv, st)
        r = work.tile([P, 1], f32, tag="lnr")
        nc.scalar.activation(r, mv[:, 1:2], AF.Sqrt, bias=epst, scale=1.0)
        r2 = work.tile([P, 1], f32, tag="lnr2")
        nc.vector.reciprocal(r2, r)
        nc.vector.tensor_scalar(
            out=xhat2[:, j, :], in0=res1[:, j, :],
            scalar1=mv[:, 0:1], scalar2=r2,
            op0=OP.subtract, op1=OP.mult,
        )

    x2T = big.tile([P, ND, CH], bf16, tag="t8a2")
    for dt_ in range(ND):
        for j in range(NO):
            pt = ptp.tile([P, P], bf16, tag="pt")
            nc.tensor.transpose(pt, xhat2[:, j, dt_ * P:(dt_ + 1) * P], ident)
            nc.scalar.copy(x2T[:, dt_, j * P:(j + 1) * P], pt)

    # ---------- FFN1 (+gelu), feature-major G^T [h, t] ----------
    w1_sb = big.tile([P, ND, HIDDEN], bf16, tag="w32a2")
    nc.sync.dma_start(out=w1_sb, in_=I["w1T"].rearrange("(i p) o -> p i o", p=P))
    g_sb = big.tile([P, NHID, CH], bf16, tag="g32")
    for ht in range(NHID):
        ps = pmm.tile([P, CH], f32, tag="ps")
        for dt_ in range(ND):
            mm(ps, w1_sb[:, dt_, ht * P:(ht + 1) * P], x2T[:, dt_, :],
               start=(dt_ == 0), stop=(dt_ == ND - 1))
        nc.scalar.activation(g_sb[:, ht, :], ps, AF.Gelu,
                             bias=b1_sb[:, ht:ht + 1], scale=1.0)

    # ---------- FFN2 + residual ----------
    w2_sb = big.tile([P, NHID, D], bf16, tag="w32b")
    nc.sync.dma_start(out=w2_sb, in_=I["w2T"].rearrange("(i p) o -> p i o", p=P))
    fin = big.tile([P, NO, D], f32, tag="t8b3")
    for tt in range(NO):
        ps = pmm.tile([P, D], f32, tag="ps")
        for ht in range(NHID):
            mm(ps, g_sb[:, ht, tt * P:(tt + 1) * P], w2_sb[:, ht, :],
               start=(ht == 0), stop=(ht == NHID - 1))
        nc.vector.tensor_add(fin[:, tt, :], ps, res1[:, tt, :])
        nc.vector.tensor_add(fin[:, tt, :], fin[:, tt, :], b2_sb)

    nc.sync.dma_start(out=y.rearrange("(j p) d -> p j d", p=P), in_=fin)


def _build():
    from contextlib import ExitStack

    import concourse.bacc as bacc
    import concourse.tile as tile
    from concourse import mybir

    f32 = mybir.dt.float32
    bf16 = mybir.dt.bfloat16
    nc = bacc.Bacc("TRN2", target_bir_lowering=False, debug=False,
                   enable_asserts=False, num_devices=NCORES)
    I = {}

    def inp(name, shape, dt_):
        I[name] = nc.dram_tensor(name, list(shape), dt_, kind="ExternalInput").ap()

    inp("xc", (T, D), f32)
    inp("wqkvT", (D, 3 * D), bf16)
    inp("bq", (P, 4), f32)
    inp("bk", (P, 4), f32)
    inp("bv", (D,), f32)
    inp("woT", (D, D), bf16)
    inp("bo", (D,), f32)
    inp("w1T", (D, HIDDEN), bf16)
    inp("b1", (P, NHID), f32)
    inp("w2T", (HIDDEN, D), bf16)
    inp("b2", (D,), f32)
    inp("masks", (3, P, P), bf16)
    y = nc.dram_tensor("y", [CH, D], f32, kind="ExternalOutput").ap()

    with tile.TileContext(nc) as tc:
        with ExitStack() as ctx:
            _body(ctx, tc, I, y)
    nc.compile()
    return nc


def _host_masks():
    import ml_dtypes
    r = np.arange(P)[:, None]
    c = np.arange(P)[None, :]
    m_lo = (c <= r).astype(np.float32)   # low k-tile: valid iff c <= r
    m_hi = (c >= r).astype(np.float32)   # high k-tile: valid iff c >= r
    m = np.stack([m_lo, m_lo, m_hi]).astype(ml_dtypes.bfloat16)
    m0 = m.copy()
    m0[0] = 0.0  # first chunk of each batch: halo keys invalid
    return np.ascontiguousarray(m), np.ascontiguousarray(m0)


def get_nc():
    global _nc
    if _nc is None:
        _nc = _build()
    return _nc


def make_in_maps(inputs):
    import ml_dtypes
    f = np.float32
    bf = ml_dtypes.bfloat16
    x = np.asarray(inputs["x"], f)
    qkv_w = np.asarray(inputs["qkv_w"], f)
    n1w = np.asarray(inputs["norm1_w"], f)
    n1b = np.asarray(inputs["norm1_b"], f)
    wqkv_f = qkv_w * n1w[None, :]
    bqkv = qkv_w @ n1b + np.asarray(inputs["qkv_b"], f)
    wqkvT = np.ascontiguousarray(wqkv_f.T.astype(bf))
    bq = np.ascontiguousarray(bqkv[0:D].reshape(4, P).T)
    bk = np.ascontiguousarray(bqkv[D:2 * D].reshape(4, P).T)
    bv = np.ascontiguousarray(bqkv[2 * D:3 * D])

    woT = np.ascontiguousarray(np.asarray(inputs["out_w"], f).T.astype(bf))
    bo = np.ascontiguousarray(np.asarray(inputs["out_b"], f))

    w1 = np.asarray(inputs["ffn_w1"], f)
    n2w = np.asarray(inputs["norm2_w"], f)
    n2b = np.asarray(inputs["norm2_b"], f)
    w1T = np.ascontiguousarray((w1 * n2w[None, :]).T.astype(bf))
    b1v = w1 @ n2b + np.asarray(inputs["ffn_b1"], f)
    b1 = np.ascontiguousarray(b1v.reshape(NHID, P).T)
    w2T = np.ascontiguousarray(np.asarray(inputs["ffn_w2"], f).T.astype(bf))
    b2 = np.ascontiguousarray(np.asarray(inputs["ffn_b2"], f))

    masks, masks0 = _host_masks()
    shared = dict(wqkvT=wqkvT, bq=bq, bk=bk, bv=bv, woT=woT, bo=bo,
                  w1T=w1T, b1=b1, w2T=w2T, b2=b2)
    in_maps = []
    for c in range(NCORES):
        b_, i = divmod(c, 4)
        own = x[b_, i * CH:(i + 1) * CH]
        if i == 0:
            halo = np.zeros((HALO, D), f)
        else:
            halo = x[b_, i * CH - HALO:i * CH]
        xc = np.ascontiguousarray(np.concatenate([halo, own], 0))
        in_maps.append(dict(xc=xc, masks=(masks if i > 0 else masks0), **shared))
    return in_maps


def kernel(**inputs):
    global LAST_EXEC_NS, LAST_RESULTS
    from concourse.bass_utils import run_bass_kernel_spmd

    nc = get_nc()
    in_maps = make_in_maps(inputs)
    trace = bool(int(os.environ.get("BASS_KERNEL_TRACE", "0")))
    res = run_bass_kernel_spmd(nc, in_maps, core_ids=list(range(NCORES)),
                               trace=trace)
    LAST_EXEC_NS = res.exec_time_ns
    LAST_RESULTS = res
    out = np.zeros((B, L, D), np.float32)
    for c, r in enumerate(res.results):
        b_, i = divmod(c, 4)
        out[b_, i * CH:(i + 1) * CH] = r["y"]
    return out


# revision 24
# speedup vs baseline: 2.7743x; 2.7743x over previous
"""Trainium2 Bass kernel for a pre-norm transformer block with dilated
windowed causal attention (B=2, L=2048, D=512, H=8, DIL=2, WIN=256,
HIDDEN=2048).

Sharding: 8 cores = batch(2) x sequence-chunk(4 x 512 tokens). Each core
receives its 512-token chunk plus a 256-token halo (keys/values only) and
computes the full block for its tokens; no collectives.

Device dataflow (per core):
  x [768,512] f32 -> LN1 (token-major stats) -> x_hat bf16 -> PE-transpose
  -> x_hat^T.  QKV (bf16 matmuls, fp32 psum): Q^T,K^T feature-major
  [hd, t]; V token-major per parity stream, ones-augmented per head.
  Attention per (head, parity stream, 128-query block): S^T[k,q] matmuls
  (dilation=2 -> two independent parity streams with a 128-wide causal
  window in stream coords), exp on ScalarE (no max subtraction: scores
  are O(1) here), band-mask multiply on GpSimd, PV with the ones row
  producing the softmax denominator, f32 reciprocal + gpsimd
  partition_broadcast for the normalization -> O^T feature-major bf16.
  out-proj -> residual (f32) -> LN2 -> FFN1+gelu -> FFN2 -> residual ->
  out [512,512] f32.

LayerNorm scale/bias are folded into the projection weights host-side;
weights are pre-transposed and cast to bf16 host-side.
"""
import os
import sys

os.environ.setdefault("MYCRO_LOCAL_CACHE", "1")
if "/opt/trn_rl_repo" not in sys.path:
    sys.path.insert(0, "/opt/trn_rl_repo")

import numpy as np

B, L, D, H, HD = 2, 2048, 512, 8, 64
HIDDEN = 4 * D
P = 128
CH = 512            # own tokens per core
HALO = 256
T = CH + HALO       # 768
NCORES = 8
EPS = 1e-5
SL = T // 2         # 384 keys per parity stream
SQ = CH // 2        # 256 queries per parity stream
SW = 128            # causal window in stream coords
SCALE = 1.0 / 8.0   # 1/sqrt(HD)

NT = T // P         # 6
NO = CH // P        # 4
ND = D // P         # 4
NHID = HIDDEN // P  # 16

_nc = None
LAST_EXEC_NS = None
LAST_RESULTS = None


def _body(ctx, tc, I, y):
    import concourse.bass as bass  # noqa: F401
    from concourse import mybir
    from concourse.masks import make_identity

    nc = tc.nc
    f32 = mybir.dt.float32
    bf16 = mybir.dt.bfloat16
    AF = mybir.ActivationFunctionType
    OP = mybir.AluOpType

    consts = ctx.enter_context(tc.tile_pool(name="consts", bufs=1))
    big = ctx.enter_context(tc.tile_pool(name="big", bufs=1))
    work = ctx.enter_context(tc.tile_pool(name="work", bufs=4))
    pmm = ctx.enter_context(tc.tile_pool(name="pmm", bufs=2, space="PSUM"))
    ptp = ctx.enter_context(tc.tile_pool(name="ptp", bufs=2, space="PSUM"))
    pa_s = ctx.enter_context(tc.tile_pool(name="pa_s", bufs=2, space="PSUM"))
    pa_o = ctx.enter_context(tc.tile_pool(name="pa_o", bufs=2, space="PSUM"))
    pexp = ctx.enter_context(tc.tile_pool(name="pexp", bufs=8))

    mm = nc.tensor.matmul

    def bcast(ap, p=P):
        return bass.AP(tensor=ap.tensor, offset=ap.offset,
                       ap=[[0, p]] + [list(d) for d in ap.ap])

    # ---------- constants ----------
    ident = consts.tile([P, P], bf16, tag="ident")
    make_identity(nc, ident)
    epst = consts.tile([P, 1], f32, tag="eps")
    nc.vector.memset(epst, EPS)
    esel = consts.tile([1, 2, P], f32, tag="esel")
    nc.vector.memset(esel, 0.0)
    nc.vector.memset(esel[0:1, 0, 0:64], 1.0)
    nc.vector.memset(esel[0:1, 1, 64:128], 1.0)

    masks_sb = consts.tile([P, 3, 2 * SQ], bf16, tag="masks")
    nc.sync.dma_start(out=masks_sb, in_=I["masks"].rearrange("k p q -> p k q"))
    bq_sb = consts.tile([P, 4], f32, tag="bq")
    nc.sync.dma_start(out=bq_sb, in_=I["bq"])
    bk_sb = consts.tile([P, 4], f32, tag="bk")
    nc.sync.dma_start(out=bk_sb, in_=I["bk"])
    b1_sb = consts.tile([P, NHID], f32, tag="b1")
    nc.sync.dma_start(out=b1_sb, in_=I["b1"])
    bv_sb = consts.tile([P, D], f32, tag="bv")
    nc.gpsimd.dma_start(out=bv_sb, in_=bcast(I["bv"]))
    bo_sb = consts.tile([P, D], f32, tag="bo")
    nc.gpsimd.dma_start(out=bo_sb, in_=bcast(I["bo"]))
    b2_sb = consts.tile([P, D], f32, tag="b2")
    nc.gpsimd.dma_start(out=b2_sb, in_=bcast(I["b2"]))

    # ---------- load x ----------
    # PE warm-up: the PE is idle ~8us while x arrives; dummy matmuls keep
    # the HAM activity window busy so real work starts at full clock.
    junk = pmm.tile([P, P], f32, tag="ps")
    for _ in range(56):
        mm(junk, ident, ident, start=True, stop=True)

    x_sb = big.tile([P, NT, D], f32, tag="x")
    nc.sync.dma_start(out=x_sb, in_=I["xc"].rearrange("(j p) d -> p j d", p=P))

    # ---------- LN1 (token-major) ----------
    xhat = big.tile([P, NT, D], bf16, tag="t12a")
    for j in range(NT):
        st = work.tile([P, 6], f32, tag="bnst")
        nc.vector.bn_stats(st, x_sb[:, j, :])
        mv = work.tile([P, 2], f32, tag="bnmv")
        nc.vector.bn_aggr(mv, st)
        r = work.tile([P, 1], f32, tag="lnr")
        nc.scalar.activation(r, mv[:, 1:2], AF.Sqrt, bias=epst, scale=1.0)
        r2 = work.tile([P, 1], f32, tag="lnr2")
        nc.vector.reciprocal(r2, r)
        nc.vector.tensor_scalar(
            out=xhat[:, j, :], in0=x_sb[:, j, :],
            scalar1=mv[:, 0:1], scalar2=r2,
            op0=OP.subtract, op1=OP.mult,
        )

    # ---------- transpose x_hat -> x_hat^T [d, t] ----------
    xT = big.tile([P, ND, T], bf16, tag="t12b")
    for dt_ in range(ND):
        for j0 in range(0, NT, 2):
            pt = ptp.tile([P, 2 * P], bf16, tag="pt")
            for jj in range(2):
                nc.tensor.transpose(pt[:, jj * P:(jj + 1) * P],
                                    xhat[:, j0 + jj, dt_ * P:(dt_ + 1) * P], ident)
            nc.scalar.copy(xT[:, dt_, j0 * P:(j0 + 2) * P], pt)

    # ---------- QKV ----------
    wqkv_sb = big.tile([P, ND, 3 * D], bf16, tag="w32a")
    nc.sync.dma_start(out=wqkv_sb, in_=I["wqkvT"].rearrange("(i p) o -> p i o", p=P))

    # Q^T [o, own t]  (own tokens only)
    qT = big.tile([P, 4, CH], bf16, tag="t8a")
    for ot in range(4):
        ps = pmm.tile([P, CH], f32, tag="ps")
        for dt_ in range(ND):
            mm(ps, wqkv_sb[:, dt_, ot * P:(ot + 1) * P], xT[:, dt_, HALO:],
               start=(dt_ == 0), stop=(dt_ == ND - 1))
        nc.scalar.activation(qT[:, ot, :], ps, AF.Identity,
                             bias=bq_sb[:, ot:ot + 1], scale=1.0)

    # K^T [o, all t] in chunks of 512+256 (one PSUM bank each)
    kT = big.tile([P, 4, T], bf16, tag="t12a2")
    for ot in range(4):
        for c0, cn in ((0, 512), (512, 256)):
            ps = pmm.tile([P, CH], f32, tag="ps")
            for dt_ in range(ND):
                mm(ps[:, :cn], wqkv_sb[:, dt_, (4 + ot) * P:(5 + ot) * P],
                   xT[:, dt_, c0:c0 + cn],
                   start=(dt_ == 0), stop=(dt_ == ND - 1))
            nc.scalar.activation(kT[:, ot, c0:c0 + cn], ps[:, :cn], AF.Identity,
                                 bias=bk_sb[:, ot:ot + 1], scale=1.0)

    # V token-major per parity stream, ones-augmented per head:
    # v_sb[:, st*3+i, h, 0:64] = V tokens, [..., 64] = 1.0
    v_sb = big.tile([P, 6, H, 65], bf16, tag="t12c")
    for i in range(6):
        nc.vector.memset(v_sb[:, i, :, 64:65], 1.0)
    for stp in range(2):
        for i in range(3):
            ps = pmm.tile([P, D], f32, tag="ps")
            t0 = 2 * (i * P) + stp
            for dt_ in range(ND):
                mm(ps, xT[:, dt_, t0:t0 + 255:2], wqkv_sb[:, dt_, 2 * D:3 * D],
                   start=(dt_ == 0), stop=(dt_ == ND - 1))
            nc.vector.tensor_add(
                v_sb[:, stp * 3 + i, :, 0:64],
                ps.rearrange("p (h c) -> p h c", h=H),
                bv_sb.rearrange("p (h c) -> p h c", h=H),
            )

    # ---------- attention ----------
    # masks_sb[:, 0] = lower-tri (c<=r) with halo validity, qb=0 low tile
    # masks_sb[:, 1] = lower-tri (c<=r), qb=1 low tile
    # masks_sb[:, 2] = upper-tri (c>=r), high tiles
    oT = big.tile([P, 4, CH], bf16, tag="t12b2")
    oU = big.tile([P, 4, CH], bf16, tag="oU")
    den4s = {}
    for hp in range(4):
        den = work.tile([97, CH], f32, tag="den")
        den4s[hp] = den
        nc.vector.memset(den, 0.0)

    def emit_S_kt(hp, hh, kt, alt):
        # kt0 is only valid for the first 128 stream-queries, kt2 only for
        # the last 128 -- their tiles are half width (qw=128 per stream).
        lo = hh * 64
        qw = SQ if kt == 1 else P
        ps_s = pa_s.tile([P, 2 * SQ], f32, tag="ps_s")
        for stp in range(2):
            k0 = 2 * (kt * P) + stp
            q0 = stp if kt < 2 else 2 * P + stp
            mm(ps_s[:, stp * qw:(stp + 1) * qw],
               kT[lo:lo + 64, hp, k0:k0 + 255:2],
               qT[lo:lo + 64, hp, q0:q0 + 2 * qw - 1:2],
               start=True, stop=True)
        p_sb = pexp.tile([P, 2 * SQ], bf16, tag="p_sb")
        nc.scalar.activation(p_sb[:, :2 * qw], ps_s[:, :2 * qw],
                             AF.Exp, scale=SCALE)
        if kt == 0:
            nc.gpsimd.tensor_mul(p_sb[:, :2 * qw], p_sb[:, :2 * qw],
                                 masks_sb[:, kt, :2 * qw])
        elif kt == 1:
            nc.vector.tensor_mul(p_sb, p_sb, masks_sb[:, kt, :])
        elif alt:
            nc.gpsimd.tensor_mul(p_sb[:, :2 * qw], p_sb[:, :2 * qw],
                                 masks_sb[:, kt, :2 * qw])
        else:
            nc.vector.tensor_mul(p_sb[:, :2 * qw], p_sb[:, :2 * qw],
                                 masks_sb[:, kt, :2 * qw])
        return p_sb

    def emit_PV(hp, hh, p_sbs):
        h = 2 * hp + hh
        lo = hh * 64
        po = pa_o.tile([P, 2 * SQ], f32, tag="po")
        for stp in range(2):
            qa = stp * SQ             # first 128 queries of this stream
            qb = stp * SQ + P         # last 128 queries
            # region A: kt0 + kt1(first half); region B: kt1(second) + kt2
            mm(po[:65, qa:qa + P], v_sb[:, stp * 3 + 0, h, :],
               p_sbs[0][:, stp * P:(stp + 1) * P], start=True, stop=False)
            mm(po[:65, qa:qa + P], v_sb[:, stp * 3 + 1, h, :],
               p_sbs[1][:, stp * SQ:stp * SQ + P], start=False, stop=True)
            mm(po[:65, qb:qb + P], v_sb[:, stp * 3 + 1, h, :],
               p_sbs[1][:, stp * SQ + P:stp * SQ + 2 * P], start=True, stop=False)
            mm(po[:65, qb:qb + P], v_sb[:, stp * 3 + 2, h, :],
               p_sbs[2][:, stp * P:(stp + 1) * P], start=False, stop=True)
        den = den4s[hp]
        for stp in range(2):
            nc.vector.tensor_copy(oU[lo:lo + 64, hp, stp::2],
                                  po[:64, stp * SQ:(stp + 1) * SQ])
            k_ = 32 * (2 * stp + hh)
            nc.vector.tensor_copy(den[k_:k_ + 1, stp::2],
                                  po[64:65, stp * SQ:(stp + 1) * SQ])

    def emit_norm(hp):
        pb = pmm.tile([P, CH], f32, tag="ps")
        mm(pb, esel, den4s[hp], start=True, stop=True)
        rb = work.tile([P, CH], f32, tag="rb")
        scr = work.tile([P, CH], f32, tag="rbscr")
        nc.vector.reciprocal_approx_accurate(rb, pb, scr)
        nc.vector.tensor_mul(oT[:, hp, :], oU[:, hp, :], rb)

    chains = [(hp, hh) for hp in range(4) for hh in range(2)]
    prev = None
    for ci, (hp, hh) in enumerate(chains):
        alt = ci % 2 == 0
        p_sbs = [emit_S_kt(hp, hh, 0, alt), emit_S_kt(hp, hh, 1, alt)]
        if prev is not None:
            emit_PV(*prev)
            if prev[1] == 1:          # second chain of prev[0] done
                emit_norm(prev[0])
        p_sbs.append(emit_S_kt(hp, hh, 2, alt))
        prev = (hp, hh, p_sbs)
    emit_PV(*prev)
    emit_norm(prev[0])

    # ---------- out projection + residual ----------
    wo_sb = big.tile([P, ND, D], bf16, tag="t8b")
    nc.sync.dma_start(out=wo_sb, in_=I["woT"])
    res1 = big.tile([P, NO, D], f32, tag="t12c2")
    for tt in range(NO):
        ps = pmm.tile([P, D], f32, tag="ps")
        for dt_ in range(ND):
            mm(ps, oT[:, dt_, tt * P:(tt + 1) * P], wo_sb[:, dt_, :],
               start=(dt_ == 0), stop=(dt_ == ND - 1))
        nc.vector.tensor_add(res1[:, tt, :], ps, x_sb[:, 2 + tt, :])
        nc.vector.tensor_add(res1[:, tt, :], res1[:, tt, :], bo_sb)

    # ---------- LN2 ----------
    xhat2 = big.tile([P, NO, D], bf16, tag="t8b2")
    for j in range(NO):
        st = work.tile([P, 6], f32, tag="bnst")
        nc.vector.bn_stats(st, res1[:, j, :])
        mv = work.tile([P, 2], f32, tag="bnmv")
        nc.vector.bn_aggr(mv, st)
        r = work.tile([P, 1], f32, tag="lnr")
        nc.scalar.activation(r, mv[:, 1:2], AF.Sqrt, bias=epst, scale=1.0)
        r2 = work.tile([P, 1], f32, tag="lnr2")
        nc.vector.reciprocal(r2, r)
        nc.vector.tensor_scalar(
            out=xhat2[:, j, :], in0=res1[:, j, :],
            scalar1=mv[:, 0:1], scalar2=r2,
            op0=OP.subtract, op1=OP.mult,
        )

    x2T = big.tile([P, ND, CH], bf16, tag="t8a2")
    for j in range(NO):
        for dt_ in range(ND):
            pt = ptp.tile([P, 2 * P], bf16, tag="pt")
            nc.tensor.transpose(pt[:, 0:P],
                                xhat2[:, j, dt_ * P:(dt_ + 1) * P], ident)
            nc.scalar.copy(x2T[:, dt_, j * P:(j + 1) * P], pt[:, 0:P])

    # ---------- FFN1 (+gelu), feature-major G^T [h, t] ----------
    w1_sb = big.tile([P, ND, HIDDEN], bf16, tag="w32a2")
    nc.sync.dma_start(out=w1_sb, in_=I["w1T"])
    g_sb = big.tile([P, NHID, CH], bf16, tag="g32")
    for ht in range(NHID):
        ps = pmm.tile([P, CH], f32, tag="ps")
        for dt_ in range(ND):
            mm(ps, w1_sb[:, dt_, ht * P:(ht + 1) * P], x2T[:, dt_, :],
               start=(dt_ == 0), stop=(dt_ == ND - 1))
        nc.scalar.activation(g_sb[:, ht, :], ps, AF.Gelu,
                             bias=b1_sb[:, ht:ht + 1], scale=1.0)

    # ---------- FFN2 + residual ----------
    w2_sb = big.tile([P, NHID, D], bf16, tag="w32b")
    nc.sync.dma_start(out=w2_sb, in_=I["w2T"])
    fin = big.tile([P, NO, D], f32, tag="t8b3")
    for tt in range(NO):
        ps = pmm.tile([P, D], f32, tag="ps")
        for ht in range(NHID):
            mm(ps, g_sb[:, ht, tt * P:(tt + 1) * P], w2_sb[:, ht, :],
               start=(ht == 0), stop=(ht == NHID - 1))
        nc.vector.tensor_add(fin[:, tt, :], ps, res1[:, tt, :])
        nc.vector.tensor_add(fin[:, tt, :], fin[:, tt, :], b2_sb)

    nc.sync.dma_start(out=y.rearrange("(j p) d -> p j d", p=P), in_=fin)


def _build():
    from contextlib import ExitStack

    import concourse.bacc as bacc
    import concourse.tile as tile
    from concourse import mybir

    f32 = mybir.dt.float32
    bf16 = mybir.dt.bfloat16
    nc = bacc.Bacc("TRN2", target_bir_lowering=False, debug=False,
                   enable_asserts=False, num_devices=NCORES)
    I = {}

    def inp(name, shape, dt_):
        I[name] = nc.dram_tensor(name, list(shape), dt_, kind="ExternalInput").ap()

    inp("xc", (P, NT, D), f32)
    inp("wqkvT", (P, ND, 3 * D), bf16)
    inp("bq", (P, 4), f32)
    inp("bk", (P, 4), f32)
    inp("bv", (D,), f32)
    inp("woT", (P, ND, D), bf16)
    inp("bo", (D,), f32)
    inp("w1T", (P, ND, HIDDEN), bf16)
    inp("b1", (P, NHID), f32)
    inp("w2T", (P, NHID, D), bf16)
    inp("b2", (D,), f32)
    inp("masks", (P, 3, 2 * SQ), bf16)
    y = nc.dram_tensor("y", [CH, D], f32, kind="ExternalOutput").ap()

    with tile.TileContext(nc) as tc:
        with ExitStack() as ctx:
            _body(ctx, tc, I, y)
    nc.compile()
    return nc


def _host_masks():
    import ml_dtypes
    sk = np.arange(SL)[:, None]
    sq = np.arange(SL - SQ, SL)[None, :]
    valid = ((sq - sk >= 0) & (sq - sk <= SW)).astype(np.float32)  # [384, 256]
    z = np.zeros((P, SQ), np.float32)
    kt0 = valid[0:P, 0:P]           # lower-tri; only first 128 queries valid
    kt1 = valid[P:2 * P, :]         # full band
    kt2 = valid[2 * P:3 * P, P:SQ]  # upper-tri; only last 128 queries valid
    m = np.stack([
        np.concatenate([kt0, kt0, z[:, :0]], 1) if False else np.concatenate([kt0, kt0, np.zeros((P, SQ), np.float32)], 1),
        np.concatenate([kt1, kt1], 1),
        np.concatenate([kt2, kt2, np.zeros((P, SQ), np.float32)], 1),
    ]).astype(ml_dtypes.bfloat16)
    m0 = m.copy()
    m0[0] = 0.0  # first chunk of each batch: halo keys invalid
    m = np.ascontiguousarray(m.transpose(1, 0, 2))
    m0 = np.ascontiguousarray(m0.transpose(1, 0, 2))
    return m, m0


def get_nc():
    global _nc
    if _nc is None:
        _nc = _build()
    return _nc


def _pmaj(a, p=P):
    """[N*p, F...] row-major -> [p, N, F...] partition-major contiguous."""
    n = a.shape[0] // p
    return np.ascontiguousarray(
        a.reshape((n, p) + a.shape[1:]).transpose((1, 0) + tuple(range(2, a.ndim + 1))))


def make_in_maps(inputs):
    import ml_dtypes
    f = np.float32
    bf = ml_dtypes.bfloat16
    x = np.asarray(inputs["x"], f)
    qkv_w = np.asarray(inputs["qkv_w"], f)
    n1w = np.asarray(inputs["norm1_w"], f)
    n1b = np.asarray(inputs["norm1_b"], f)
    wqkv_f = qkv_w * n1w[None, :]
    bqkv = qkv_w @ n1b + np.asarray(inputs["qkv_b"], f)
    wqkvT = _pmaj(np.ascontiguousarray(wqkv_f.T).astype(bf))
    bq = np.ascontiguousarray(bqkv[0:D].reshape(4, P).T)
    bk = np.ascontiguousarray(bqkv[D:2 * D].reshape(4, P).T)
    bv = np.ascontiguousarray(bqkv[2 * D:3 * D])

    woT = _pmaj(np.ascontiguousarray(np.asarray(inputs["out_w"], f).T).astype(bf))
    bo = np.ascontiguousarray(np.asarray(inputs["out_b"], f))

    w1 = np.asarray(inputs["ffn_w1"], f)
    n2w = np.asarray(inputs["norm2_w"], f)
    n2b = np.asarray(inputs["norm2_b"], f)
    w1T = _pmaj(np.ascontiguousarray((w1 * n2w[None, :]).T).astype(bf))
    b1v = w1 @ n2b + np.asarray(inputs["ffn_b1"], f)
    b1 = np.ascontiguousarray(b1v.reshape(NHID, P).T)
    w2T = _pmaj(np.ascontiguousarray(np.asarray(inputs["ffn_w2"], f).T).astype(bf))
    b2 = np.ascontiguousarray(np.asarray(inputs["ffn_b2"], f))

    masks, masks0 = _host_masks()
    shared = dict(wqkvT=wqkvT, bq=bq, bk=bk, bv=bv, woT=woT, bo=bo,
                  w1T=w1T, b1=b1, w2T=w2T, b2=b2)
    in_maps = []
    for c in range(NCORES):
        b_, i = divmod(c, 4)
        own = x[b_, i * CH:(i + 1) * CH]
        if i == 0:
            halo = np.zeros((HALO, D), f)
        else:
            halo = x[b_, i * CH - HALO:i * CH]
        xc = _pmaj(np.concatenate([halo, own], 0))
        in_maps.append(dict(xc=xc, masks=(masks if i > 0 else masks0), **shared))
    return in_maps


def kernel(**inputs):
    global LAST_EXEC_NS, LAST_RESULTS
    from concourse.bass_utils import run_bass_kernel_spmd

    nc = get_nc()
    in_maps = make_in_maps(inputs)
    trace = bool(int(os.environ.get("BASS_KERNEL_TRACE", "0")))
    res = run_bass_kernel_spmd(nc, in_maps, core_ids=list(range(NCORES)),
                               trace=trace)
    LAST_EXEC_NS = res.exec_time_ns
    LAST_RESULTS = res
    out = np.zeros((B, L, D), np.float32)
    for c, r in enumerate(res.results):
        b_, i = divmod(c, 4)
        out[b_, i * CH:(i + 1) * CH] = r["y"]
    return out
